# revision 12
# baseline (speedup 1.0000x reference)
"""Trainium2 Bass kernel for nn_ConcatSquashLinearSA3.

Strategy: data-parallel across batch B — each of the 8 cores owns 4
batches (full N=2048).  The channel-attention mix is folded into
per-batch weights on device:

    P_b = W_layer^T (I - A2_b) W_tc^T          [128(i) x 256(e')]
    q_b = b_layer  (I - A2_b) W_tc^T + b_tc    [256]
so  t_b = x_b P_b + q_b   and   x1_b = x_b W_layer^T + b_layer.

The FiLM gate/bias are folded too:  out = x1gh + relu(bn(t)) * g_bc
with x1gh = x_b (W_layer^T diag g_b) + (b_layer*g_b + h_b).

All per-n work runs in [n-on-partitions, e-on-free] layout, so BN
scale/shift are per-partition ACT scale/bias operands and the per-n
batch statistics come from free-dim sums (ACT accum_out / DVE reduce).
Two batches are packed side-by-side in every [128, 512] tile (same
n-chunk => same BN scalars), halving elementwise op count.  BN stats
are summed across cores with a 16 KB DRAM AllReduce.  Output is bf16,
cast to f32 on host.
"""

import os
import sys

sys.path.insert(0, "/opt/trn_rl_repo")

import numpy as np
import ml_dtypes

import concourse.bass as bass
import concourse.bacc as bacc
import concourse.mybir as mybir
import concourse.tile as tile
from concourse.bass_utils import run_bass_kernel_spmd

F32 = mybir.dt.float32
F32R = mybir.dt.float32r
BF16 = mybir.dt.bfloat16

B, N, DIN, DOUT, DCTX = 32, 2048, 128, 256, 259
NCORES = 8
BLOC = B // NCORES          # 4 batches per core
NPAIR = BLOC // 2           # 2 batch-pairs per core
NCHUNK = N // 128           # 16 n-chunks of 128 rows
BN_EPS = 1e-5

Act = mybir.ActivationFunctionType
Alu = mybir.AluOpType

_cached = {}


def build_program(reps=1, use_cc=True):
    nc = bacc.Bacc("TRN2", target_bir_lowering=False, debug=False,
                   num_devices=NCORES)

    xT = nc.dram_tensor("xT", [BLOC, 128, N], BF16, kind="ExternalInput").ap()
    wlt = nc.dram_tensor("WlT", [128, 256], BF16, kind="ExternalInput").ap()
    wl = nc.dram_tensor("Wl", [2, 128, 128], BF16, kind="ExternalInput").ap()
    wtct = nc.dram_tensor("WtcT", [2, 128, 256], BF16,
                          kind="ExternalInput").ap()
    wgh = nc.dram_tensor("Wgh", [3, 128, 512], F32, kind="ExternalInput").ap()
    wk = nc.dram_tensor("Wk", [3, 128, 256], F32, kind="ExternalInput").ap()
    wv = nc.dram_tensor("Wv", [3, 128, 256], F32, kind="ExternalInput").ap()
    ctxt = nc.dram_tensor("ctxT", [3, 128, BLOC], F32,
                          kind="ExternalInput").ap()
    rows = nc.dram_tensor("rows", [1, 512], F32, kind="ExternalInput").ap()
    blc = nc.dram_tensor("blc", [128, 2], BF16, kind="ExternalInput").ap()
    gb = nc.dram_tensor("gb", [128, 32], F32, kind="ExternalInput").ap()
    onesr = nc.dram_tensor("onesr", [1, 128], F32, kind="ExternalInput").ap()
    outB = nc.dram_tensor("outB", [reps, NPAIR, 128, NCHUNK, 512], BF16,
                          kind="ExternalOutput").ap()

    import contextlib
    with tile.TileContext(nc) as tc:
        with contextlib.ExitStack() as ctx:
            p_const = ctx.enter_context(tc.tile_pool(name="const", bufs=1))
            p_perb = ctx.enter_context(tc.tile_pool(name="perb", bufs=2 * BLOC))
            p_tsb = ctx.enter_context(
                tc.tile_pool(name="tsb", bufs=NPAIR * NCHUNK))
            p_scr = ctx.enter_context(tc.tile_pool(name="scr", bufs=10))
            p_obuf = ctx.enter_context(tc.tile_pool(name="obuf", bufs=6))
            p_stats = ctx.enter_context(tc.tile_pool(name="stats", bufs=2))
            p_small = ctx.enter_context(tc.tile_pool(name="small", bufs=2))
            p_tiny = ctx.enter_context(tc.tile_pool(name="tiny", bufs=8))
            ps_t = ctx.enter_context(
                tc.tile_pool(name="ps_t", bufs=3, space="PSUM"))
            ps_x = ctx.enter_context(
                tc.tile_pool(name="ps_x", bufs=3, space="PSUM"))
            ps_s = ctx.enter_context(
                tc.tile_pool(name="ps_s", bufs=2, space="PSUM"))
            p_dram = ctx.enter_context(
                tc.tile_pool(name="dram", bufs=4, space="DRAM"))
            for r in range(reps):
                # each rep writes its own outB slice so no rep is dead code
                _emit(nc, tc, r, use_cc,
                      xT, wlt, wl, wtct, wgh, wk, wv, ctxt, rows, blc, gb,
                      onesr, outB[r],
                      p_const, p_perb, p_tsb, p_scr, p_obuf, p_stats,
                      p_small, p_tiny, ps_t, ps_x, ps_s, p_dram)

    nc.compile()
    return nc


def _emit(nc, tc, r, use_cc, xT, wlt, wl, wtct, wgh, wk, wv, ctxt, rows, blc,
          gb, onesr, outB, p_const, p_perb, p_tsb, p_scr, p_obuf, p_stats,
          p_small, p_tiny, ps_t, ps_x, ps_s, p_dram):
    # ---- load constants (ctx-path weights first so per-batch prep can
    # start while the big xT loads stream in on the scalar HWDGE ring) ----
    c_wgh, c_wk, c_wv, c_ctxt = [], [], [], []
    for ct in range(3):
        t = p_const.tile([128, BLOC], F32R, tag=f"ctxt{ct}",
                         name=f"ctxt{ct}_{r}")
        nc.sync.dma_start(t[:], ctxt[ct].bitcast(F32R))
        c_ctxt.append(t)
    for ct in range(3):
        t = p_const.tile([128, 256], F32R, tag=f"wk{ct}", name=f"wk{ct}_{r}")
        nc.sync.dma_start(t[:], wk[ct].bitcast(F32R))
        c_wk.append(t)
        t = p_const.tile([128, 256], F32R, tag=f"wv{ct}", name=f"wv{ct}_{r}")
        nc.sync.dma_start(t[:], wv[ct].bitcast(F32R))
        c_wv.append(t)
        t = p_const.tile([128, 512], F32R, tag=f"wgh{ct}", name=f"wgh{ct}_{r}")
        nc.sync.dma_start(t[:], wgh[ct].bitcast(F32R))
        c_wgh.append(t)
    c_xT = []
    for b in range(BLOC):
        t = p_const.tile([128, N], BF16, tag=f"xT{b}", name=f"xT{b}_{r}")
        nc.scalar.dma_start(t[:], xT[b])
        c_xT.append(t)
    c_wl = []
    for oc in range(2):
        t = p_const.tile([128, 128], BF16, tag=f"wl{oc}", name=f"wl{oc}_{r}")
        nc.sync.dma_start(t[:], wl[oc])
        c_wl.append(t)
    c_wtct = []
    for ec in range(2):
        t = p_const.tile([128, 256], BF16, tag=f"wtct{ec}",
                         name=f"wtct{ec}_{r}")
        nc.sync.dma_start(t[:], wtct[ec])
        c_wtct.append(t)
    c_wlt = p_const.tile([128, 256], BF16, tag="wlt", name=f"wlt{r}")
    nc.sync.dma_start(c_wlt[:], wlt[:])
    c_rows = p_const.tile([1, 512], F32, tag="rows", name=f"rows{r}")
    nc.sync.dma_start(c_rows[:], rows[:])
    c_blc = p_const.tile([128, 2], BF16, tag="blc", name=f"blc{r}")
    nc.sync.dma_start(c_blc[:], blc[:])
    c_gb = p_const.tile([128, 32], F32, tag="gb", name=f"gb{r}")
    nc.sync.dma_start(c_gb[:], gb[:])
    c_onesr = p_const.tile([1, 128], F32R, tag="onesr", name=f"onesr{r}")
    nc.sync.dma_start(c_onesr[:], onesr[:].bitcast(F32R))
    ones_bf = p_const.tile([1, 128], BF16, tag="ones_bf", name=f"onbf{r}")
    nc.vector.memset(ones_bf[:], 1.0)
    ones16 = p_const.tile([128, 1], BF16, tag="ones16", name=f"on16{r}")
    nc.vector.memset(ones16[:], 1.0)

    _ps_pools = [ps_s, ps_t, ps_x]
    _ps_tags = ["psA", "t_ps", "x1_ps"]
    _ps_idx = [0]

    def psA(name, p, f):
        i = _ps_idx[0] % 3
        _ps_idx[0] += 1
        t = _ps_pools[i].tile([128, 512], F32, tag=_ps_tags[i], name=name)
        return t[0:p, 0:f]

    # ---- ctx-derived rows (gate | hbias) and cols (k), v rows ----
    rows_ps = psA(f"rowsps{r}", BLOC, 512)
    for ct in range(3):
        nc.tensor.matmul(rows_ps[:], c_ctxt[ct][:], c_wgh[ct][:],
                         start=(ct == 0), stop=(ct == 2))
    grow = p_small.tile([BLOC, 256], F32, tag="grow", name=f"grow{r}")
    nc.scalar.activation(grow[:], rows_ps[:, 0:256], Act.Sigmoid)
    hrow = p_small.tile([BLOC, 256], F32, tag="hrow", name=f"hrow{r}")
    nc.vector.tensor_scalar(hrow[:], rows_ps[:, 256:512], 0.0, None, Alu.add)

    v_ps = psA(f"vps{r}", BLOC, 256)
    for ct in range(3):
        nc.tensor.matmul(v_ps[:], c_ctxt[ct][:], c_wv[ct][:],
                         start=(ct == 0), stop=(ct == 2))
    vrow = p_small.tile([BLOC, 256], F32, tag="vrow", name=f"vrow{r}")
    nc.vector.tensor_scalar(vrow[:], v_ps[:], 0.0, None, Alu.add)

    kcol = []
    for oc in range(2):
        kc_ps = psA(f"kcps{oc}_{r}", 128, BLOC)
        for ct in range(3):
            nc.tensor.matmul(kc_ps[:], c_wk[ct][:, oc * 128:(oc + 1) * 128],
                             c_ctxt[ct][:], start=(ct == 0), stop=(ct == 2))
        kc = p_small.tile([128, BLOC], F32, tag=f"kcol{oc}",
                          name=f"kcol{oc}_{r}")
        nc.vector.tensor_scalar(kc[:], kc_ps[:], 0.0, None, Alu.add)
        kcol.append(kc)

    # flatten rows to partition 0 (K=1 matmul rhs needs base partition 0)
    gfl = p_small.tile([1, BLOC * 256], F32R, tag="gfl", name=f"gfl{r}")
    nc.sync.dma_start(gfl[:].rearrange("p (b x) -> p b x", b=BLOC),
                      grow[:].bitcast(F32R))
    hfl = p_small.tile([1, BLOC * 256], F32, tag="hfl", name=f"hfl{r}")
    nc.sync.dma_start(hfl[:].rearrange("p (b x) -> p b x", b=BLOC), hrow[:])
    vfl = p_small.tile([1, BLOC * 256], F32R, tag="vfl", name=f"vfl{r}")
    nc.sync.dma_start(vfl[:].rearrange("p (b x) -> p b x", b=BLOC),
                      vrow[:].bitcast(F32R))

    # ---- G0 = WlT @ WtcT (f32 in SBUF), q0 row ----
    g0_ps = psA(f"g0ps{r}", 128, 256)
    for oc in range(2):
        nc.tensor.matmul(g0_ps[:], c_wl[oc][:], c_wtct[oc][:],
                         start=(oc == 0), stop=(oc == 1))
    G0 = p_small.tile([128, 256], F32, tag="G0", name=f"G0_{r}")
    nc.vector.tensor_scalar(G0[:], g0_ps[:], 0.0, None, Alu.add)

    q0_ps = psA(f"q0ps{r}", 1, 256)
    for oc in range(2):
        nc.tensor.matmul(q0_ps[:], c_blc[:, oc:oc + 1], c_wtct[oc][:],
                         start=(oc == 0), stop=(oc == 1))
    q0row = p_small.tile([1, 256], F32, tag="q0row", name=f"q0row{r}")
    nc.vector.tensor_tensor(q0row[:], q0_ps[:], c_rows[0:1, 256:512], Alu.add)

    # ---- per-batch P / Wg prep; per-pair q2/blgh2/gbc2 ----
    P_b, Wg_b = [], []
    qrow2, blgh2, gbc2 = [], [], []
    for b in range(BLOC):
        bp, half = divmod(b, 2)
        if half == 0:
            qrow2.append(p_small.tile([1, 512], BF16, tag="qrow2",
                                      bufs=2 * NPAIR, name=f"qrow2_{bp}_{r}"))
            blgh2.append(p_small.tile([1, 512], BF16, tag="blgh2",
                                      bufs=2 * NPAIR, name=f"blgh2_{bp}_{r}"))
            gbc2.append(p_perb.tile([128, 512], BF16, tag="gbc2",
                                    bufs=2 * NPAIR, name=f"gbc2_{bp}_{r}"))
        hs = slice(half * 256, (half + 1) * 256)

        vbc = psA(f"vbc{b}_{r}", 128, 256)
        nc.tensor.matmul(vbc[:], c_onesr[:],
                         vfl[0:1, b * 256:(b + 1) * 256],
                         start=True, stop=True)
        attn = []
        for oc in range(2):
            att = p_perb.tile([128, 256], BF16, tag="att",
                              name=f"att{b}{oc}_{r}")
            Z = p_tiny.tile([128, 1], F32, tag="Z", name=f"Z{b}{oc}_{r}")
            nc.scalar.activation(att[:], vbc[:], Act.Exp,
                                 scale=kcol[oc][:, b:b + 1], accum_out=Z[:])
            rZ = p_tiny.tile([128, 1], F32, tag="rZ", name=f"rZ{b}{oc}_{r}")
            nc.vector.reciprocal(rZ[:], Z[:])
            an = p_perb.tile([128, 256], BF16, tag="attn",
                             name=f"attn{b}{oc}_{r}")
            nc.scalar.activation(an[:], att[:], Act.Identity, scale=rZ[:])
            attn.append(an)

        rcol = []
        for ec in range(2):
            c_ps = psA(f"cps{b}{ec}_{r}", 128, 1)
            for oc in range(2):
                nc.tensor.matmul(c_ps[:],
                                 attn[oc][:, ec * 128:(ec + 1) * 128],
                                 ones16[:], start=(oc == 0), stop=(oc == 1))
            rc = p_tiny.tile([128, 1], F32, tag="rcol", name=f"rc{b}{ec}_{r}")
            nc.vector.tensor_scalar(rc[:], c_ps[:], 1e-9, None, Alu.add)
            nc.vector.reciprocal(rc[:], rc[:])
            rcol.append(rc)

        ut2 = []
        for ec in range(2):
            ut_ps = psA(f"utps{b}{ec}_{r}", 128, 128)
            for oc in range(2):
                nc.tensor.matmul(ut_ps[:],
                                 attn[oc][:, ec * 128:(ec + 1) * 128],
                                 c_wl[oc][:], start=(oc == 0), stop=(oc == 1))
            u2 = p_perb.tile([128, 128], BF16, tag="ut2",
                             name=f"ut2{b}{ec}_{r}")
            nc.vector.tensor_scalar(u2[:], ut_ps[:], rcol[ec][:], None,
                                    Alu.mult)
            ut2.append(u2)

        p_ps = psA(f"pps{b}_{r}", 128, 256)
        for ec in range(2):
            nc.tensor.matmul(p_ps[:], ut2[ec][:], c_wtct[ec][:],
                             start=(ec == 0), stop=(ec == 1))
        P = p_perb.tile([128, 256], BF16, tag="P", name=f"P{b}_{r}")
        nc.vector.tensor_tensor(P[:], G0[:], p_ps[:], Alu.subtract)
        P_b.append(P)

        w2 = []
        for ec in range(2):
            w_ps = psA(f"wps{b}{ec}_{r}", 128, 1)
            for oc in range(2):
                nc.tensor.matmul(w_ps[:],
                                 attn[oc][:, ec * 128:(ec + 1) * 128],
                                 c_blc[:, oc:oc + 1],
                                 start=(oc == 0), stop=(oc == 1))
            w2t = p_tiny.tile([128, 1], BF16, tag="w2", name=f"w2{b}{ec}_{r}")
            nc.vector.tensor_tensor(w2t[:], w_ps[:], rcol[ec][:], Alu.mult)
            w2.append(w2t)
        q1_ps = psA(f"q1ps{b}_{r}", 1, 256)
        for ec in range(2):
            nc.tensor.matmul(q1_ps[:], w2[ec][:], c_wtct[ec][:],
                             start=(ec == 0), stop=(ec == 1))
        nc.vector.tensor_tensor(qrow2[bp][0:1, hs], q0row[:], q1_ps[:],
                                Alu.subtract)

        gbc_ps = psA(f"gbcps{b}_{r}", 128, 256)
        nc.tensor.matmul(gbc_ps[:], c_onesr[:],
                         gfl[0:1, b * 256:(b + 1) * 256],
                         start=True, stop=True)
        nc.vector.tensor_scalar(gbc2[bp][:, hs], gbc_ps[:], 0.0, None,
                                Alu.add)
        wg = p_perb.tile([128, 256], BF16, tag="wg", name=f"wg{b}_{r}")
        nc.vector.tensor_tensor(wg[:], c_wlt[:], gbc_ps[:], Alu.mult)
        Wg_b.append(wg)

        bt = p_tiny.tile([1, 256], F32, tag="blghf", name=f"blghf{b}_{r}")
        nc.vector.tensor_tensor(bt[:],
                                gfl[0:1, b * 256:(b + 1) * 256].bitcast(F32),
                                c_rows[0:1, 0:256], Alu.mult)
        nc.vector.tensor_tensor(blgh2[bp][0:1, hs], bt[:],
                                hfl[0:1, b * 256:(b + 1) * 256], Alu.add)

    # ---- pass 1: t2 = [x_b0 P_b0 | x_b1 P_b1] + q2, stats ----
    ssum = p_stats.tile([128, 32], F32, tag="ssum", bufs=1, name=f"ssum{r}")
    ssq = p_stats.tile([128, 32], F32, tag="ssq", bufs=1, name=f"ssq{r}")
    t_sb = [[None] * NCHUNK for _ in range(NPAIR)]
    for bp in range(NPAIR):
        for c in range(NCHUNK):
            idx = bp * NCHUNK + c
            b0, b1 = 2 * bp, 2 * bp + 1
            t_ps = ps_t.tile([128, 512], F32, tag="t_ps",
                             name=f"tps{bp}_{c}_{r}")
            cs = slice(c * 128, (c + 1) * 128)
            nc.tensor.matmul(t_ps[:, 0:256], c_xT[b0][:, cs], P_b[b0][:],
                             start=True, stop=False)
            nc.tensor.matmul(t_ps[:, 256:512], c_xT[b1][:, cs], P_b[b1][:],
                             start=False, stop=False)
            nc.tensor.matmul(t_ps[:], ones_bf[:], qrow2[bp][:],
                             start=False, stop=True)
            ts = p_tsb.tile([128, 512], BF16, tag="t_sb",
                            name=f"tsb{bp}_{c}_{r}")
            t_sb[bp][c] = ts
            sq = p_scr.tile([128, 512], BF16, tag="sq",
                            name=f"sq{bp}_{c}_{r}")
            if idx % 3 == 0:
                # cast on DVE (+DVE sum-reduce), square on ACT (+accum)
                nc.vector.tensor_scalar(ts[:], t_ps[:], 0.0, None, Alu.add)
                nc.vector.tensor_reduce(ssum[:, idx:idx + 1], ts[:],
                                        axis=mybir.AxisListType.X, op=Alu.add)
                nc.scalar.activation(sq[:], ts[:], Act.Square,
                                     accum_out=ssq[:, idx:idx + 1])
            else:
                # cast on ACT (+accum), square on Pool (+DVE sum-reduce)
                nc.scalar.activation(ts[:], t_ps[:], Act.Identity,
                                     accum_out=ssum[:, idx:idx + 1])
                nc.gpsimd.tensor_tensor(sq[:], ts[:], ts[:], Alu.mult)
                nc.vector.tensor_reduce(ssq[:, idx:idx + 1], sq[:],
                                        axis=mybir.AxisListType.X, op=Alu.add)

    # ---- stats reduce over local pairs + AllReduce ----
    stloc = p_stats.tile([128, 32], F32, tag="stloc", bufs=2, name=f"stl{r}")
    for kind, st_src in ((0, ssum), (1, ssq)):
        v = st_src[:].rearrange("p (b c) -> p b c", b=NPAIR)
        dst = stloc[:, kind * 16:(kind + 1) * 16]
        nc.vector.tensor_tensor(dst, v[:, 0, :], v[:, 1, :], Alu.add)

    stg = p_stats.tile([128, 32], F32, tag="stg", bufs=2, name=f"stg{r}")
    if use_cc:
        ib = p_dram.tile([128, 32], F32, name=f"ccin{r}")
        ob = p_dram.tile([128, 32], F32, name=f"ccout{r}")
        nc.sync.dma_start(ib[:], stloc[:])
        nc.gpsimd.collective_compute(
            "AllReduce", Alu.add,
            replica_groups=[list(range(NCORES))],
            ins=[ib.opt()], outs=[ob.opt()])
        nc.sync.dma_start(stg[:], ob[:])
    else:
        # sim-profiling mode: no collective (stats wrong by 8x scale but
        # timing-representative); emulate with a DRAM bounce.
        ib = p_dram.tile([128, 32], F32, name=f"ccin{r}")
        nc.sync.dma_start(ib[:], stloc[:])
        nc.sync.dma_start(stg[:], ib[:])

    # ---- BN finalize: s = gamma*istd, sh = beta - mean*s  [128, 16] ----
    inv_n = 1.0 / (B * DOUT)
    mean = p_small.tile([128, 16], F32, tag="mean", name=f"mean{r}")
    nc.vector.tensor_scalar(mean[:], stg[:, 0:16], inv_n, None, Alu.mult)
    ex2 = p_small.tile([128, 16], F32, tag="ex2", name=f"ex2{r}")
    nc.vector.tensor_scalar(ex2[:], stg[:, 16:32], inv_n, None, Alu.mult)
    var = p_small.tile([128, 16], F32, tag="var", name=f"var{r}")
    nc.vector.tensor_tensor(var[:], mean[:], mean[:], Alu.mult)
    nc.vector.tensor_tensor(var[:], ex2[:], var[:], Alu.subtract)
    nc.vector.tensor_scalar(var[:], var[:], BN_EPS, None, Alu.add)
    std = p_small.tile([128, 16], F32, tag="std", name=f"std{r}")
    nc.scalar.activation(std[:], var[:], Act.Sqrt)
    istd = p_small.tile([128, 16], F32, tag="istd", name=f"istd{r}")
    nc.vector.reciprocal(istd[:], std[:])
    s_all = p_small.tile([128, 16], F32, tag="s_all", name=f"sall{r}")
    nc.vector.tensor_tensor(s_all[:], istd[:], c_gb[:, 0:16], Alu.mult)
    sh_all = p_small.tile([128, 16], F32, tag="sh_all", name=f"shall{r}")
    nc.vector.tensor_tensor(sh_all[:], mean[:], s_all[:], Alu.mult)
    nc.vector.tensor_tensor(sh_all[:], c_gb[:, 16:32], sh_all[:],
                            Alu.subtract)

    # ---- pass 2: out = [x Wg]2 + blgh2 + relu(t2*s+sh)*gbc2 ----
    for bp in range(NPAIR):
        b0, b1 = 2 * bp, 2 * bp + 1
        for cg in range(NCHUNK // 2):
            obuf = p_obuf.tile([128, 2, 512], BF16, tag="obuf",
                               name=f"ob{bp}_{cg}_{r}")
            for cc in range(2):
                c = cg * 2 + cc
                idx = bp * NCHUNK + c
                cs = slice(c * 128, (c + 1) * 128)
                if idx % 2 == 0:
                    x1_ps = ps_x.tile([128, 512], F32, tag="x1_ps",
                                      name=f"x1ps{bp}_{c}_{r}")
                else:
                    x1_ps = ps_t.tile([128, 512], F32, tag="t_ps",
                                      name=f"x1ps{bp}_{c}_{r}")
                nc.tensor.matmul(x1_ps[:, 0:256], c_xT[b0][:, cs],
                                 Wg_b[b0][:], start=True, stop=False)
                nc.tensor.matmul(x1_ps[:, 256:512], c_xT[b1][:, cs],
                                 Wg_b[b1][:], start=False, stop=False)
                nc.tensor.matmul(x1_ps[:], ones_bf[:], blgh2[bp][:],
                                 start=False, stop=True)
                zg = p_scr.tile([128, 512], BF16, tag="zg",
                                name=f"zg{bp}_{c}_{r}")
                zr = p_scr.tile([128, 512], BF16, tag="zr",
                                name=f"zr{bp}_{c}_{r}")
                nc.scalar.activation(zr[:], t_sb[bp][c][:], Act.Relu,
                                     scale=s_all[:, c:c + 1],
                                     bias=sh_all[:, c:c + 1])
                if idx % 3 != 0:
                    nc.gpsimd.tensor_tensor(zg[:], zr[:], gbc2[bp][:],
                                            Alu.mult)
                else:
                    nc.vector.tensor_tensor(zg[:], zr[:], gbc2[bp][:],
                                            Alu.mult)
                nc.vector.tensor_tensor(obuf[:, cc, :], zg[:], x1_ps[:],
                                        Alu.add)
            nc.scalar.dma_start(outB[bp][:, cg * 2:(cg + 1) * 2, :], obuf[:])


def _prep_inputs(ctx, x, W_layer, b_layer, W_hbias, W_gate, b_gate,
                 W_k, W_v, W_tc, b_tc, bn_gamma, bn_beta):
    """Host-side shard + layout prep.  Returns list of 8 in_maps."""
    x = np.asarray(x, np.float32)
    ctx = np.asarray(ctx, np.float32).reshape(B, DCTX)

    WlT16 = np.ascontiguousarray(
        np.asarray(W_layer, np.float32).T).astype(ml_dtypes.bfloat16)
    Wl16 = np.ascontiguousarray(
        np.asarray(W_layer, np.float32).reshape(2, 128, 128)
    ).astype(ml_dtypes.bfloat16)
    WtcT16 = np.ascontiguousarray(
        np.asarray(W_tc, np.float32).T.reshape(2, 128, 256)
    ).astype(ml_dtypes.bfloat16)

    # gate|hbias weights with b_gate folded in via the ctx ones-row (row 259)
    Wgh = np.zeros((384, 512), np.float32)
    Wgh[:DCTX, 0:256] = np.asarray(W_gate, np.float32).T
    Wgh[DCTX, 0:256] = np.asarray(b_gate, np.float32)
    Wgh[:DCTX, 256:512] = np.asarray(W_hbias, np.float32).T
    Wgh = np.ascontiguousarray(Wgh.reshape(3, 128, 512))

    def padk(w):                                       # [o, 256] -> [384, 256]
        out = np.zeros((384, 256), np.float32)
        out[:256] = np.asarray(w, np.float32).T
        return np.ascontiguousarray(out.reshape(3, 128, 256))

    Wk3 = padk(W_k)
    Wv3 = padk(W_v)

    rows = np.zeros((1, 512), np.float32)
    rows[0, 0:256] = np.asarray(b_layer, np.float32)
    rows[0, 256:512] = np.asarray(b_tc, np.float32)
    blc16 = np.ascontiguousarray(
        np.asarray(b_layer, np.float32).reshape(2, 128).T
    ).astype(ml_dtypes.bfloat16)

    gbm = np.empty((128, 32), np.float32)
    gbm[:, 0:16] = np.asarray(bn_gamma, np.float32).reshape(16, 128).T
    gbm[:, 16:32] = np.asarray(bn_beta, np.float32).reshape(16, 128).T

    in_maps = []
    for c in range(NCORES):
        bset = slice(c * BLOC, (c + 1) * BLOC)
        xTc = np.ascontiguousarray(
            x[bset].transpose(0, 2, 1)).astype(ml_dtypes.bfloat16)
        ctxT = np.zeros((384, BLOC), np.float32)
        ctxT[:DCTX] = ctx[bset].T
        ctxT[DCTX] = 1.0
        in_maps.append({
            "xT": xTc, "WlT": WlT16, "Wl": Wl16, "WtcT": WtcT16,
            "Wgh": Wgh, "Wk": Wk3, "Wv": Wv3,
            "ctxT": np.ascontiguousarray(ctxT.reshape(3, 128, BLOC)),
            "rows": rows, "blc": blc16, "gb": gbm,
            "onesr": np.ones((1, 128), np.float32),
        })
    return in_maps


def kernel(**inputs):
    if "nc" not in _cached:
        _cached["nc"] = build_program()
    nc = _cached["nc"]
    in_maps = _prep_inputs(**inputs)
    res = run_bass_kernel_spmd(nc, in_maps, core_ids=list(range(NCORES)),
                               trace=bool(int(os.environ.get("KTRACE", "0"))))
    _cached["last_result"] = res
    out = np.empty((B, N, DOUT), np.float32)
    for c in range(NCORES):
        ob = np.asarray(res.results[c]["outB"][0], dtype=np.float32)
        # [NPAIR, 128(p), 16(chunk), 512]; free col h*256+e is batch 2*bp+h
        for bp in range(NPAIR):
            for h in range(2):
                b = c * BLOC + 2 * bp + h
                out[b] = ob[bp, :, :, h * 256:(h + 1) * 256].transpose(
                    1, 0, 2).reshape(N, DOUT)
    return out
